# revision 39
# baseline (speedup 1.0000x reference)
"""Trainium2 Bass kernel for nn_MultiHeadAttention (B=4, S=2048, D=1024, H=16).

Sharding: 8 cores = 4 batches x 2 query-halves. Each core computes full K/V
projections for its batch (keys are permuted so the core's own queries come
first), attention for its 1024 queries over all 2048 keys, and the output
projection for its query half. No collectives needed.

Per-core dataflow:
  x arrives pre-transposed from host as xT in fp8-e4m3 (for the K/V
  DoubleRow projections) and bf16 (for the Q projection, which stays bf16:
  fp8 noise on q+k together would breach the error budget). K/V weights
  arrive pre-scaled by 32 in fp8 (uniform(+-1/32) entries otherwise land in
  e4m3 subnormals); Wq pre-scaled by 32 in bf16. Scale bookkeeping:
    qT,kp store 32q,32k (bf16);  scores psum = 1024 q.k;  exp scale = 2^-13
    v3 stores 32(v+bv) fp8 (bias folded);  ones-column = 32 so the
    denominator row is 32*sum(ex);  oT = o_num * recip(den) = o + bv.
  Attention: per head pair, scores via row-paired K=64 bf16 matmuls; exp on
  ACT into fp8 pair-tiles ex[128, 2, 1024] (two kt chunks); AV runs as
  DoubleRow fp8 matmuls contracting both kt chunks at once (2x PE).
  Output projection in bf16: y = oT @ Wo + bo.
"""

import numpy as np
import ml_dtypes
from contextlib import ExitStack

import concourse.bass as bass
from concourse import bacc
import concourse.mybir as mybir
import concourse.tile as tile
from concourse.bass_utils import run_bass_kernel_spmd

F32 = mybir.dt.float32
BF16 = mybir.dt.bfloat16
FP8 = mybir.dt.float8e4
AF = mybir.ActivationFunctionType
DR = mybir.MatmulPerfMode.DoubleRow
NPBF16 = ml_dtypes.bfloat16
NPFP8 = ml_dtypes.float8_e4m3

P = 128
WSCALE = 32.0

# precision config (set per CPU-simulated error budget; gate is 2e-2)
K_FP8 = True     # K projection as fp8 DoubleRow
V_FP8 = False    # V projection fp8 costs 1.8e-2 alone - stays bf16
DR_AV = False    # attn@V as fp8 DoubleRow (ex+v3 in fp8) - costs ~1.8e-2 alone
AV_DT = mybir.dt.float8e4 if DR_AV else mybir.dt.bfloat16

N_CORES = 8
B_FULL, S_FULL, D_FULL = 4, 2048, 1024
H_FULL, DH = 16, 64


def build_mha_nc(S=2048, Sq=1024, D=1024, H=16, scale=None):
    """Build the per-core Bass program. Returns nc."""
    assert D % P == 0 and S % P == 0 and Sq % P == 0 and H % 2 == 0
    ND = D // P            # d-tiles
    NDP = ND // 2          # DoubleRow d-tile pairs
    NS = S // P            # s-chunks / k-tiles
    NSP = NS // 2          # kt chunk pairs for DoubleRow AV
    NPAIR = H // 2
    W65 = DH + 1           # augmented head width (v | 32s)
    QSP = min(512, Sq)     # q span
    NQS = Sq // QSP
    KSP = min(512, S)      # span for kT projection
    NKS = S // KSP
    CSP = min(512, D)      # col span for v / out projections
    NCS = D // CSP
    HPS = CSP // DH        # heads per col-span in v projection
    if scale is None:
        scale = DH ** -0.5
    act_scale = float(scale / (WSCALE * WSCALE))  # 2^-13 exact

    nc = bacc.Bacc(target_bir_lowering=False, debug=False)

    # span-major layouts: [P][span][j][s-within-span], so each 512-span of
    # all 8 j-tiles is one contiguous 4KB-per-partition DMA
    xT = nc.dram_tensor("xT", [P, NKS, ND, KSP], FP8, kind="ExternalInput").ap()
    XB_LEN = Sq if (K_FP8 and V_FP8) else S
    NBS = XB_LEN // KSP
    xTb_d = nc.dram_tensor("xTb", [P, NBS, ND, KSP], BF16, kind="ExternalInput").ap()
    W = {"Wk": nc.dram_tensor("Wk", [P, ND * D], FP8 if K_FP8 else BF16,
                              kind="ExternalInput").ap(),
         "Wv": nc.dram_tensor("Wv", [P, ND * D], FP8 if V_FP8 else BF16,
                              kind="ExternalInput").ap()}
    Wq_d = nc.dram_tensor("Wq", [P, ND * D], FP8, kind="ExternalInput").ap()
    Wo_d = nc.dram_tensor("Wo", [P, ND * D], BF16, kind="ExternalInput").ap()
    bias = {n: nc.dram_tensor(n, [D], F32, kind="ExternalInput").ap()
            for n in ("bq", "bk", "bv", "bo")}
    y = nc.dram_tensor("y", [Sq, D], F32, kind="ExternalOutput").ap()

    with tile.TileContext(nc) as tc, ExitStack() as top:
        top.enter_context(nc.allow_low_precision(
            reason="fp8/bf16 activations+weights with fp32 psum accumulation"))
        const = top.enter_context(tc.tile_pool(name="const", bufs=1))
        big = top.enter_context(tc.tile_pool(name="big", bufs=1))
        wp = top.enter_context(tc.tile_pool(name="wp", bufs=1))
        kpool = top.enter_context(tc.tile_pool(name="kpool", bufs=3))
        ppsL = top.enter_context(tc.tile_pool(name="ppsL", bufs=1, space="PSUM"))

        # per-partition bias layouts: b_sb[p, j] = 32*b[j*128 + p]
        bq_sb = const.tile([P, ND], F32)
        nc.gpsimd.dma_start(out=bq_sb, in_=bias["bq"].rearrange("(j p) -> p j", p=P))
        bk_sb = const.tile([P, ND], F32)
        nc.gpsimd.dma_start(out=bk_sb, in_=bias["bk"].rearrange("(j p) -> p j", p=P))
        # bv (x32) / bo broadcast across partitions (small DMA + on-chip bcast)
        bv_bc = const.tile([P, D], F32)
        bo_bc = const.tile([P, D], F32)
        with tc.tile_pool(name="btmp", bufs=1) as btmp:
            brow = btmp.tile([1, D], F32, tag="brow", name="bv_row")
            nc.gpsimd.dma_start(out=brow, in_=bias["bv"].unsqueeze(0))
            nc.gpsimd.partition_broadcast(bv_bc, brow, channels=P)
            brow2 = btmp.tile([1, D], F32, tag="brow", name="bo_row")
            nc.gpsimd.dma_start(out=brow2, in_=bias["bo"].unsqueeze(0))
            nc.gpsimd.partition_broadcast(bo_bc, brow2, channels=P)

        oT = big.tile([P, ND, Sq], BF16)
        xTs = big.tile([P, NKS, ND, KSP], FP8)      # span-major: contig DMA
        xTb = big.tile([P, NBS, ND, KSP], BF16)
        qTs = big.tile([P, ND, Sq], BF16)
        v_sb = big.tile([P, NS, H * W65], AV_DT)
        v3 = v_sb.rearrange("p i (h w) -> p i h w", w=W65)

        # ones-column (value 32) for the denominator row of v_aug
        nc.vector.memset(v3[:, :, :, DH:DH + 1], WSCALE)

        # arrival order matches early compute order: qT -> v(0..11) -> kT;
        # every x chunk is a fully contiguous per-partition DMA
        Wq_sb = wp.tile([P, ND, D], FP8, tag="wq")
        nc.sync.dma_start(out=Wq_sb, in_=Wq_d.rearrange("p (j c) -> p j c", j=ND))
        Wo_sb = wp.tile([P, ND, D], BF16, tag="wo")
        for spn in range(NQS):
            nc.sync.dma_start(out=xTs[:, spn], in_=xT[:, spn])
        Wk_sb = wp.tile([P, ND, D], FP8 if K_FP8 else BF16, tag="wk")
        nc.sync.dma_start(out=Wk_sb, in_=W["Wk"].rearrange("p (j c) -> p j c", j=ND))
        for spn in range(NQS, NKS):
            nc.sync.dma_start(out=xTs[:, spn], in_=xT[:, spn])
        Wv_sb = wp.tile([P, ND, D], FP8 if V_FP8 else BF16, tag="wv")
        nc.sync.dma_start(out=Wv_sb, in_=W["Wv"].rearrange("p (j c) -> p j c", j=ND))
        for spn in range(NBS):
            nc.sync.dma_start(out=xTb[:, spn], in_=xTb_d[:, spn])
        nc.sync.dma_start(out=Wo_sb, in_=Wo_d.rearrange("p (j c) -> p j c", j=ND))

        def qT_span(dc, sp, pool):
            ps = pool.tile([P, QSP], F32, tag="pp0", name=f"qps_{dc}_{sp}")
            for jj in range(NDP):
                nc.tensor.matmul(
                    ps,
                    Wq_sb[:, 2 * jj:2 * jj + 2, dc * P:(dc + 1) * P],
                    xTs[:, sp, 2 * jj:2 * jj + 2, :],
                    start=(jj == 0), stop=(jj == NDP - 1),
                    perf_mode=DR,
                )
            nc.vector.tensor_scalar_add(
                qTs[:, dc, sp * QSP:(sp + 1) * QSP], ps, bq_sb[:, dc:dc + 1])

        def v_span(i, sp, pool):
            ps = pool.tile([P, CSP], F32, tag="pp0", name=f"vps_{i}_{sp}")
            iq, ir = divmod(i, KSP // P)
            if V_FP8:
                for jj in range(NDP):
                    nc.tensor.matmul(
                        ps,
                        xTs[:, iq, 2 * jj:2 * jj + 2, ir * P:(ir + 1) * P],
                        Wv_sb[:, 2 * jj:2 * jj + 2, sp * CSP:(sp + 1) * CSP],
                        start=(jj == 0), stop=(jj == NDP - 1),
                        perf_mode=DR,
                    )
            else:
                for j in range(ND):
                    nc.tensor.matmul(
                        ps,
                        xTb[:, iq, j, ir * P:(ir + 1) * P],
                        Wv_sb[:, j, sp * CSP:(sp + 1) * CSP],
                        start=(j == 0), stop=(j == ND - 1),
                    )
            nc.vector.tensor_add(
                v3[:, i, sp * HPS:(sp + 1) * HPS, 0:DH],
                ps.rearrange("p (h w) -> p h w", w=DH),
                bv_bc[:, sp * CSP:(sp + 1) * CSP].rearrange("p (h w) -> p h w", w=DH),
            )

        def kT_span(kp, p, sp, pool):
            ps = pool.tile([P, KSP], F32, tag="pp0", name=f"kps_{p}_{sp}")
            if K_FP8:
                for jj in range(NDP):
                    nc.tensor.matmul(
                        ps,
                        Wk_sb[:, 2 * jj:2 * jj + 2, p * P:(p + 1) * P],
                        xTs[:, sp, 2 * jj:2 * jj + 2, :],
                        start=(jj == 0), stop=(jj == NDP - 1),
                        perf_mode=DR,
                    )
            else:
                for j in range(ND):
                    nc.tensor.matmul(
                        ps,
                        Wk_sb[:, j, p * P:(p + 1) * P],
                        xTb[:, sp, j, :],
                        start=(j == 0), stop=(j == ND - 1),
                    )
            nc.vector.tensor_scalar_add(
                kp[:, sp * KSP:(sp + 1) * KSP], ps, bk_sb[:, p:p + 1])

        def oproj_span(sc_i, csp, pool, stg):
            ps = pool.tile([P, CSP], F32, tag="pp0", name=f"yps_{sc_i}_{csp}")
            for j in range(ND):
                nc.tensor.matmul(
                    ps,
                    oT[:, j, sc_i * P:(sc_i + 1) * P],
                    Wo_sb[:, j, csp * CSP:(csp + 1) * CSP],
                    start=(j == 0), stop=(j == ND - 1),
                )
            ysb = stg.tile([P, CSP], F32, tag="ysb", name=f"ysb_{sc_i}_{csp}")
            nc.vector.tensor_add(ysb, ps, bo_bc[:, csp * CSP:(csp + 1) * CSP])
            nc.sync.dma_start(
                out=y[sc_i * P:(sc_i + 1) * P, csp * CSP:(csp + 1) * CSP],
                in_=ysb,
            )

        # ---- Early phase: qT, v(first half), kp0/kp1 dense on the PE;
        # v(NS/2..NS-1, 0) lands inside pair 0 just ahead of its use ----
        kps = []
        with tc.tile_pool(name="ppsE", bufs=2, space="PSUM") as ppsE:
            for dc in range(2):
                for s in range(NQS):
                    qT_span(dc, s, ppsE)
            # K projections fill the PE while xTb/Wv are still arriving
            for p012 in range(3):
                kp = kpool.tile([P, S], BF16, tag="kp", name=f"kp_{p012}")
                kps.append(kp)
                for s in range(NKS):
                    kT_span(kp, p012, s, ppsE)
            for s in range(NQS):
                qT_span(2, s, ppsE)
            for i in range(3 * NS // 4):
                v_span(i, 0, ppsE)

        # ---- Attention: projection/output side-work emitted one small
        # span-group per kt-pair slot (strict-FIFO PE queue: big blocks would
        # starve the scalar engine) ----
        with tc.tile_pool(name="exp", bufs=2) as exq, \
             tc.tile_pool(name="eps", bufs=2) as eps, \
             tc.tile_pool(name="ystg", bufs=1) as ystg, \
             tc.tile_pool(name="scps", bufs=2, space="PSUM") as scps, \
             tc.tile_pool(name="ops", bufs=2, space="PSUM") as opsum:
            def emit_sc_act(kp, p, qsl, kt, expair, t):
                sc = scps.tile([P, 2 * QSP], F32, tag="sc",
                               name=f"sc_{p}_{kt}_{t}")
                nc.tensor.matmul(
                    sc[:, 0:QSP],
                    kp[0:DH, kt * P:(kt + 1) * P],
                    qTs[0:DH, p, qsl],
                    start=True, stop=True,
                )
                nc.tensor.matmul(
                    sc[:, QSP:2 * QSP],
                    kp[DH:P, kt * P:(kt + 1) * P],
                    qTs[DH:P, p, qsl],
                    start=True, stop=True,
                )
                nc.scalar.activation(expair[:, t, :], sc, AF.Exp, scale=act_scale)

            phases = [(p, sp) for p in range(NPAIR) for sp in range(NQS)]
            carry = None  # expair tile holding next phase's (kt=0, t=0) exp
            for idx, (p, sp) in enumerate(phases):
                kp = kps[p]
                qsl = slice(sp * QSP, (sp + 1) * QSP)
                # v(i,1) spans owed by pair 4; spread over pairs 1-3
                V1 = {(1, 0): (0, 3), (1, 1): (3, 6), (2, 0): (6, 9),
                      (2, 1): (9, 12), (3, 0): (12, 14), (3, 1): (14, 16)}
                # side work to scatter into this (pair, sp)'s slack slots;
                # each list must fit the ~7.6us ACT slack of one (p, sp)
                work = []
                if 1 <= p and p + 2 < NPAIR:
                    if sp == 0:
                        kp2 = kpool.tile([P, S], BF16, tag="kp",
                                         name=f"kp_{p + 2}")
                        kps.append(kp2)
                        kplast = kp2
                        work += [lambda: qT_span(p + 2, 0, ppsL)]
                        work += [(lambda s=s, k=kp2: kT_span(k, p + 2, s, ppsL))
                                 for s in range(2)]
                    else:
                        work += [lambda: qT_span(p + 2, 1, ppsL)]
                        work += [(lambda s=s, k=kplast: kT_span(k, p + 2, s, ppsL))
                                 for s in range(2, NKS)]
                if sp == 0 and p == 0:
                    # last quarter of v(sp0): lands just ahead of its use
                    work += [(lambda i=i: v_span(i, 0, ppsL))
                             for i in range(3 * NS // 4, NS)]
                if NCS > 1 and (p, sp) in V1:
                    lo, hi = V1[(p, sp)]
                    work += [(lambda i=i: v_span(i, 1, ppsL))
                             for i in range(lo, hi)]
                if p == NPAIR - 1 and sp == 1:
                    work += [(lambda si=si, c=c: oproj_span(si, c, ppsL, ystg))
                             for si in range(Sq // (2 * P))
                             for c in range(NCS)]
                o_even = opsum.tile([W65, QSP], F32, tag="op",
                                    name=f"oe_{p}_{sp}")
                o_odd = opsum.tile([W65, QSP], F32, tag="op",
                                   name=f"oo_{p}_{sp}")
                for ktp in range(NSP):
                    if ktp == 0 and carry is not None:
                        expair = carry
                        carry = None
                        ts = (1,)          # (kt=0, t=0) already emitted
                    else:
                        expair = exq.tile([P, 2, 2 * QSP], AV_DT, tag="ex",
                                          name=f"ex_{p}_{sp}_{ktp}")
                        ts = (0, 1)
                    for t in ts:
                        emit_sc_act(kp, p, qsl, 2 * ktp + t, expair, t)
                    if DR_AV:
                        nc.tensor.matmul(
                            o_even,
                            v3[:, 2 * ktp:2 * ktp + 2, 2 * p, :],
                            expair[:, :, 0:QSP],
                            start=(ktp == 0), stop=(ktp == NSP - 1),
                            perf_mode=DR,
                        )
                        nc.tensor.matmul(
                            o_odd,
                            v3[:, 2 * ktp:2 * ktp + 2, 2 * p + 1, :],
                            expair[:, :, QSP:2 * QSP],
                            start=(ktp == 0), stop=(ktp == NSP - 1),
                            perf_mode=DR,
                        )
                    else:
                        for t in range(2):
                            nc.tensor.matmul(
                                o_even,
                                v3[:, 2 * ktp + t, 2 * p, :],
                                expair[:, t, 0:QSP],
                                start=(ktp == 0 and t == 0),
                                stop=(ktp == NSP - 1 and t == 1),
                            )
                            nc.tensor.matmul(
                                o_odd,
                                v3[:, 2 * ktp + t, 2 * p + 1, :],
                                expair[:, t, QSP:2 * QSP],
                                start=(ktp == 0 and t == 0),
                                stop=(ktp == NSP - 1 and t == 1),
                            )
                    if work:
                        work.pop(0)()
                # prologue of the next phase: its first scores+exp go ahead of
                # our epilogue so the scalar engine never idles at the boundary
                if idx + 1 < len(phases):
                    np_, nsp = phases[idx + 1]
                    nqsl = slice(nsp * QSP, (nsp + 1) * QSP)
                    carry = exq.tile([P, 2, 2 * QSP], AV_DT, tag="ex",
                                     name=f"ex_{np_}_{nsp}_0")
                    emit_sc_act(kps[np_], np_, nqsl, 0, carry, 0)
                # epilogue: oT = o_num * recip(den)  (= o + bv, scales cancel)
                for par, ops in ((0, o_even), (1, o_odd)):
                    den0 = eps.tile([1, QSP], F32, tag="den0")
                    nc.vector.tensor_copy(den0, ops[DH:W65, :])
                    rc0 = eps.tile([1, QSP], F32, tag="rc0")
                    nc.vector.reciprocal_approx_fast(rc0, den0)
                    rb = eps.tile([DH, QSP], F32, tag="rb_sb")
                    nc.gpsimd.partition_broadcast(rb, rc0, channels=DH)
                    if par == 0:
                        nc.vector.tensor_mul(
                            oT[0:DH, p, qsl], ops[0:DH, :], rb)
                    else:
                        on = eps.tile([DH, QSP], BF16, tag="on")
                        nc.vector.tensor_mul(on, ops[0:DH, :], rb)
                        nc.sync.dma_start(out=oT[DH:P, p, qsl], in_=on)
                    if work:
                        work.pop(0)()
                for w in work:
                    w()

        # ---- Output projection tail (second query half) ----
        with tc.tile_pool(name="ystg2", bufs=4) as ystg2, \
             tc.tile_pool(name="yps", bufs=2, space="PSUM") as yps:
            for sc_i in range(Sq // (2 * P), Sq // P):
                pss = [yps.tile([P, CSP], F32, tag=f"yp{csp}",
                                name=f"yps_{sc_i}_{csp}") for csp in range(NCS)]
                for j in range(ND):
                    for csp in range(NCS):
                        nc.tensor.matmul(
                            pss[csp],
                            oT[:, j, sc_i * P:(sc_i + 1) * P],
                            Wo_sb[:, j, csp * CSP:(csp + 1) * CSP],
                            start=(j == 0), stop=(j == ND - 1),
                        )
                for csp in range(NCS):
                    ysb = ystg2.tile([P, CSP], F32, tag="ysb")
                    nc.vector.tensor_add(ysb, pss[csp],
                                         bo_bc[:, csp * CSP:(csp + 1) * CSP])
                    nc.sync.dma_start(
                        out=y[sc_i * P:(sc_i + 1) * P, csp * CSP:(csp + 1) * CSP],
                        in_=ysb,
                    )

    nc.compile()
    return nc


_NC = None


def _get_nc():
    global _NC
    if _NC is None:
        _NC = build_mha_nc(S=S_FULL, Sq=S_FULL // 2, D=D_FULL, H=H_FULL)
    return _NC


def _to_fp8(a):
    return np.clip(np.asarray(a, np.float32), -240.0, 240.0).astype(NPFP8)


def shard_inputs(inputs):
    x = np.asarray(inputs["x"], dtype=np.float32)
    shared = {}
    for n, f in (("Wk", K_FP8), ("Wv", V_FP8), ("Wq", True)):
        w = 32.0 * np.asarray(inputs[n], dtype=np.float32)
        # W_sb[p, j, c] = 32*W[j*128+p, c], flattened to [128, 8*1024]
        w = _to_fp8(w) if f else w.astype(NPBF16)
        shared[n] = np.ascontiguousarray(
            w.reshape(8, P, D_FULL).transpose(1, 0, 2).reshape(P, -1))
    wo = np.asarray(inputs["Wo"], dtype=np.float32)
    shared["Wo"] = np.ascontiguousarray(
        wo.astype(NPBF16).reshape(8, P, D_FULL).transpose(1, 0, 2).reshape(P, -1))
    shared["bq"] = np.ascontiguousarray(32.0 * np.asarray(inputs["bq"], np.float32))
    shared["bk"] = np.ascontiguousarray(32.0 * np.asarray(inputs["bk"], np.float32))
    shared["bv"] = np.ascontiguousarray(32.0 * np.asarray(inputs["bv"], np.float32))
    shared["bo"] = np.ascontiguousarray(np.asarray(inputs["bo"], np.float32))
    half = S_FULL // 2
    maps = []
    for c in range(N_CORES):
        b, h = divmod(c, 2)
        xb = x[b]
        xp = np.concatenate([xb[h * half:(h + 1) * half],
                             xb[(1 - h) * half:(2 - h) * half]], axis=0)
        # xT[p, spn, j, u] = xp[spn*512+u, j*128+p]  (span-major per partition)
        xpT = xp.T
        xt = (_to_fp8(xpT).reshape(8, P, 4, 512)
              .transpose(1, 2, 0, 3).reshape(P, 4, 8, 512))
        xb_len = S_FULL // 2 if (K_FP8 and V_FP8) else S_FULL
        xtb = (xpT[:, :xb_len].astype(NPBF16)
               .reshape(8, P, xb_len // 512, 512)
               .transpose(1, 2, 0, 3).reshape(P, xb_len // 512, 8, 512))
        m = dict(shared)
        m["xT"] = np.ascontiguousarray(xt)
        m["xTb"] = np.ascontiguousarray(xtb)
        maps.append(m)
    return maps


def run(inputs, trace=False):
    nc = _get_nc()
    maps = shard_inputs(inputs)
    res = run_bass_kernel_spmd(nc, maps, list(range(N_CORES)), trace=trace)
    half = S_FULL // 2
    y = np.empty((B_FULL, S_FULL, D_FULL), dtype=np.float32)
    for c in range(N_CORES):
        b, h = divmod(c, 2)
        y[b, h * half:(h + 1) * half] = res.results[c]["y"]
    return y, res


def kernel(**inputs):
    y, _ = run(inputs, trace=False)
    return y


# revision 41
# speedup vs baseline: 1.0027x; 1.0027x over previous
"""Trainium2 Bass kernel for nn_MultiHeadAttention (B=4, S=2048, D=1024, H=16).

Sharding: 8 cores = 4 batches x 2 query-halves. Each core computes full K/V
projections for its batch (keys are permuted so the core's own queries come
first), attention for its 1024 queries over all 2048 keys, and the output
projection for its query half. No collectives needed.

Per-core dataflow:
  x arrives pre-transposed from host as xT in fp8-e4m3 (for the K/V
  DoubleRow projections) and bf16 (for the Q projection, which stays bf16:
  fp8 noise on q+k together would breach the error budget). K/V weights
  arrive pre-scaled by 32 in fp8 (uniform(+-1/32) entries otherwise land in
  e4m3 subnormals); Wq pre-scaled by 32 in bf16. Scale bookkeeping:
    qT,kp store 32q,32k (bf16);  scores psum = 1024 q.k;  exp scale = 2^-13
    v3 stores 32(v+bv) fp8 (bias folded);  ones-column = 32 so the
    denominator row is 32*sum(ex);  oT = o_num * recip(den) = o + bv.
  Attention: per head pair, scores via row-paired K=64 bf16 matmuls; exp on
  ACT into fp8 pair-tiles ex[128, 2, 1024] (two kt chunks); AV runs as
  DoubleRow fp8 matmuls contracting both kt chunks at once (2x PE).
  Output projection in bf16: y = oT @ Wo + bo.
"""

import numpy as np
import ml_dtypes
from contextlib import ExitStack

import concourse.bass as bass
from concourse import bacc
import concourse.mybir as mybir
import concourse.tile as tile
from concourse.bass_utils import run_bass_kernel_spmd

F32 = mybir.dt.float32
BF16 = mybir.dt.bfloat16
FP8 = mybir.dt.float8e4
AF = mybir.ActivationFunctionType
DR = mybir.MatmulPerfMode.DoubleRow
NPBF16 = ml_dtypes.bfloat16
NPFP8 = ml_dtypes.float8_e4m3

P = 128
WSCALE = 32.0

# precision config (set per CPU-simulated error budget; gate is 2e-2)
K_FP8 = True     # K projection as fp8 DoubleRow
V_FP8 = False    # V projection fp8 costs 1.8e-2 alone - stays bf16
DR_AV = False    # attn@V as fp8 DoubleRow (ex+v3 in fp8) - costs ~1.8e-2 alone
AV_DT = mybir.dt.float8e4 if DR_AV else mybir.dt.bfloat16

N_CORES = 8
B_FULL, S_FULL, D_FULL = 4, 2048, 1024
H_FULL, DH = 16, 64


def build_mha_nc(S=2048, Sq=1024, D=1024, H=16, scale=None):
    """Build the per-core Bass program. Returns nc."""
    assert D % P == 0 and S % P == 0 and Sq % P == 0 and H % 2 == 0
    ND = D // P            # d-tiles
    NDP = ND // 2          # DoubleRow d-tile pairs
    NS = S // P            # s-chunks / k-tiles
    NSP = NS // 2          # kt chunk pairs for DoubleRow AV
    NPAIR = H // 2
    W65 = DH + 1           # augmented head width (v | 32s)
    QSP = min(512, Sq)     # q span
    NQS = Sq // QSP
    KSP = min(512, S)      # span for kT projection
    NKS = S // KSP
    CSP = min(512, D)      # col span for v / out projections
    NCS = D // CSP
    HPS = CSP // DH        # heads per col-span in v projection
    if scale is None:
        scale = DH ** -0.5
    act_scale = float(scale / (WSCALE * WSCALE))  # 2^-13 exact

    nc = bacc.Bacc(target_bir_lowering=False, debug=False)

    # span-major layouts: [P][span][j][s-within-span], so each 512-span of
    # all 8 j-tiles is one contiguous 4KB-per-partition DMA
    xT = nc.dram_tensor("xT", [P, NKS, ND, KSP], FP8, kind="ExternalInput").ap()
    XB_LEN = Sq if (K_FP8 and V_FP8) else S
    NBS = XB_LEN // KSP
    xTb_d = nc.dram_tensor("xTb", [P, NBS, ND, KSP], BF16, kind="ExternalInput").ap()
    W = {"Wk": nc.dram_tensor("Wk", [P, ND * D], FP8 if K_FP8 else BF16,
                              kind="ExternalInput").ap(),
         "Wv": nc.dram_tensor("Wv", [P, ND * D], FP8 if V_FP8 else BF16,
                              kind="ExternalInput").ap()}
    Wq_d = nc.dram_tensor("Wq", [P, ND * D], FP8, kind="ExternalInput").ap()
    Wo_d = nc.dram_tensor("Wo", [P, ND * D], BF16, kind="ExternalInput").ap()
    bias = {n: nc.dram_tensor(n, [D], F32, kind="ExternalInput").ap()
            for n in ("bq", "bk", "bv", "bo")}
    y = nc.dram_tensor("y", [Sq, D], F32, kind="ExternalOutput").ap()

    with tile.TileContext(nc) as tc, ExitStack() as top:
        top.enter_context(nc.allow_low_precision(
            reason="fp8/bf16 activations+weights with fp32 psum accumulation"))
        const = top.enter_context(tc.tile_pool(name="const", bufs=1))
        big = top.enter_context(tc.tile_pool(name="big", bufs=1))
        wp = top.enter_context(tc.tile_pool(name="wp", bufs=1))
        kpool = top.enter_context(tc.tile_pool(name="kpool", bufs=3))
        ppsL = top.enter_context(tc.tile_pool(name="ppsL", bufs=1, space="PSUM"))

        # per-partition bias layouts: b_sb[p, j] = 32*b[j*128 + p]
        bq_sb = const.tile([P, ND], F32)
        nc.gpsimd.dma_start(out=bq_sb, in_=bias["bq"].rearrange("(j p) -> p j", p=P))
        bk_sb = const.tile([P, ND], F32)
        nc.gpsimd.dma_start(out=bk_sb, in_=bias["bk"].rearrange("(j p) -> p j", p=P))
        # bv (x32) / bo broadcast across partitions (small DMA + on-chip bcast)
        bv_bc = const.tile([P, D], F32)
        bo_bc = const.tile([P, D], F32)
        with tc.tile_pool(name="btmp", bufs=1) as btmp:
            brow = btmp.tile([1, D], F32, tag="brow", name="bv_row")
            nc.gpsimd.dma_start(out=brow, in_=bias["bv"].unsqueeze(0))
            nc.gpsimd.partition_broadcast(bv_bc, brow, channels=P)
            brow2 = btmp.tile([1, D], F32, tag="brow", name="bo_row")
            nc.gpsimd.dma_start(out=brow2, in_=bias["bo"].unsqueeze(0))
            nc.gpsimd.partition_broadcast(bo_bc, brow2, channels=P)

        oT = big.tile([P, ND, Sq], BF16)
        xTs = big.tile([P, NKS, ND, KSP], FP8)      # span-major: contig DMA
        xTb = big.tile([P, NBS, ND, KSP], BF16)
        qTs = big.tile([P, ND, Sq], BF16)
        v_sb = big.tile([P, NS, H * W65], AV_DT)
        v3 = v_sb.rearrange("p i (h w) -> p i h w", w=W65)

        # ones-column (value 32) for the denominator row of v_aug
        nc.vector.memset(v3[:, :, :, DH:DH + 1], WSCALE)

        # arrival order matches early compute order: qT -> v(0..11) -> kT;
        # every x chunk is a fully contiguous per-partition DMA
        Wq_sb = wp.tile([P, ND, D], FP8, tag="wq")
        nc.sync.dma_start(out=Wq_sb, in_=Wq_d.rearrange("p (j c) -> p j c", j=ND))
        Wo_sb = wp.tile([P, ND, D], BF16, tag="wo")
        for spn in range(NQS):
            nc.sync.dma_start(out=xTs[:, spn], in_=xT[:, spn])
        Wk_sb = wp.tile([P, ND, D], FP8 if K_FP8 else BF16, tag="wk")
        nc.sync.dma_start(out=Wk_sb, in_=W["Wk"].rearrange("p (j c) -> p j c", j=ND))
        for spn in range(NQS, NKS):
            nc.sync.dma_start(out=xTs[:, spn], in_=xT[:, spn])
        Wv_sb = wp.tile([P, ND, D], FP8 if V_FP8 else BF16, tag="wv")
        nc.sync.dma_start(out=Wv_sb, in_=W["Wv"].rearrange("p (j c) -> p j c", j=ND))
        for spn in range(NBS):
            nc.sync.dma_start(out=xTb[:, spn], in_=xTb_d[:, spn])
        nc.sync.dma_start(out=Wo_sb, in_=Wo_d.rearrange("p (j c) -> p j c", j=ND))

        def qT_span(dc, sp, pool):
            ps = pool.tile([P, QSP], F32, tag="pp0", name=f"qps_{dc}_{sp}")
            for jj in range(NDP):
                nc.tensor.matmul(
                    ps,
                    Wq_sb[:, 2 * jj:2 * jj + 2, dc * P:(dc + 1) * P],
                    xTs[:, sp, 2 * jj:2 * jj + 2, :],
                    start=(jj == 0), stop=(jj == NDP - 1),
                    perf_mode=DR,
                )
            nc.vector.tensor_scalar_add(
                qTs[:, dc, sp * QSP:(sp + 1) * QSP], ps, bq_sb[:, dc:dc + 1])

        def v_span(i, sp, pool):
            ps = pool.tile([P, CSP], F32, tag="pp0", name=f"vps_{i}_{sp}")
            iq, ir = divmod(i, KSP // P)
            if V_FP8:
                for jj in range(NDP):
                    nc.tensor.matmul(
                        ps,
                        xTs[:, iq, 2 * jj:2 * jj + 2, ir * P:(ir + 1) * P],
                        Wv_sb[:, 2 * jj:2 * jj + 2, sp * CSP:(sp + 1) * CSP],
                        start=(jj == 0), stop=(jj == NDP - 1),
                        perf_mode=DR,
                    )
            else:
                for j in range(ND):
                    nc.tensor.matmul(
                        ps,
                        xTb[:, iq, j, ir * P:(ir + 1) * P],
                        Wv_sb[:, j, sp * CSP:(sp + 1) * CSP],
                        start=(j == 0), stop=(j == ND - 1),
                    )
            nc.vector.tensor_add(
                v3[:, i, sp * HPS:(sp + 1) * HPS, 0:DH],
                ps.rearrange("p (h w) -> p h w", w=DH),
                bv_bc[:, sp * CSP:(sp + 1) * CSP].rearrange("p (h w) -> p h w", w=DH),
            )

        def kT_span(kp, p, sp, pool):
            ps = pool.tile([P, KSP], F32, tag="pp0", name=f"kps_{p}_{sp}")
            if K_FP8:
                for jj in range(NDP):
                    nc.tensor.matmul(
                        ps,
                        Wk_sb[:, 2 * jj:2 * jj + 2, p * P:(p + 1) * P],
                        xTs[:, sp, 2 * jj:2 * jj + 2, :],
                        start=(jj == 0), stop=(jj == NDP - 1),
                        perf_mode=DR,
                    )
            else:
                for j in range(ND):
                    nc.tensor.matmul(
                        ps,
                        Wk_sb[:, j, p * P:(p + 1) * P],
                        xTb[:, sp, j, :],
                        start=(j == 0), stop=(j == ND - 1),
                    )
            nc.vector.tensor_scalar_add(
                kp[:, sp * KSP:(sp + 1) * KSP], ps, bk_sb[:, p:p + 1])

        def oproj_span(sc_i, csp, pool, stg):
            ps = pool.tile([P, CSP], F32, tag="pp0", name=f"yps_{sc_i}_{csp}")
            for j in range(ND):
                nc.tensor.matmul(
                    ps,
                    oT[:, j, sc_i * P:(sc_i + 1) * P],
                    Wo_sb[:, j, csp * CSP:(csp + 1) * CSP],
                    start=(j == 0), stop=(j == ND - 1),
                )
            ysb = stg.tile([P, CSP], F32, tag="ysb", name=f"ysb_{sc_i}_{csp}")
            nc.vector.tensor_add(ysb, ps, bo_bc[:, csp * CSP:(csp + 1) * CSP])
            nc.sync.dma_start(
                out=y[sc_i * P:(sc_i + 1) * P, csp * CSP:(csp + 1) * CSP],
                in_=ysb,
            )

        # ---- Early phase: qT, v(first half), kp0/kp1 dense on the PE;
        # v(NS/2..NS-1, 0) lands inside pair 0 just ahead of its use ----
        kps = []
        with tc.tile_pool(name="ppsE", bufs=2, space="PSUM") as ppsE:
            for dc in range(2):
                for s in range(NQS):
                    qT_span(dc, s, ppsE)
            # K projections fill the PE while xTb/Wv are still arriving
            for p012 in range(3):
                kp = kpool.tile([P, S], BF16, tag="kp", name=f"kp_{p012}")
                kps.append(kp)
                for s in range(NKS):
                    kT_span(kp, p012, s, ppsE)
            for s in range(NQS):
                qT_span(2, s, ppsE)
            for i in range(3 * NS // 4):
                v_span(i, 0, ppsE)

        # ---- Attention: projection/output side-work emitted one small
        # span-group per kt-pair slot (strict-FIFO PE queue: big blocks would
        # starve the scalar engine) ----
        with tc.tile_pool(name="exp", bufs=2) as exq, \
             tc.tile_pool(name="eps", bufs=2) as eps, \
             tc.tile_pool(name="ystg", bufs=1) as ystg, \
             tc.tile_pool(name="scps", bufs=2, space="PSUM") as scps, \
             tc.tile_pool(name="ops", bufs=2, space="PSUM") as opsum:
            def emit_sc_act(kp, p, qsl, kt, expair, t):
                sc = scps.tile([P, 2 * QSP], F32, tag="sc",
                               name=f"sc_{p}_{kt}_{t}")
                nc.tensor.matmul(
                    sc[:, 0:QSP],
                    kp[0:DH, kt * P:(kt + 1) * P],
                    qTs[0:DH, p, qsl],
                    start=True, stop=True,
                )
                nc.tensor.matmul(
                    sc[:, QSP:2 * QSP],
                    kp[DH:P, kt * P:(kt + 1) * P],
                    qTs[DH:P, p, qsl],
                    start=True, stop=True,
                )
                nc.scalar.activation(expair[:, t, :], sc, AF.Exp, scale=act_scale)

            phases = [(p, sp) for p in range(NPAIR) for sp in range(NQS)]
            carry = None  # expair tile holding next phase's (kt=0, t=0) exp
            for idx, (p, sp) in enumerate(phases):
                kp = kps[p]
                qsl = slice(sp * QSP, (sp + 1) * QSP)
                # v(i,1) spans owed by pair 4; spread over pairs 1-3
                V1 = {(1, 0): (0, 3), (1, 1): (3, 6), (2, 0): (6, 9),
                      (2, 1): (9, 12), (3, 0): (12, 14), (3, 1): (14, 16)}
                # side work to scatter into this (pair, sp)'s slack slots;
                # each list must fit the ~7.6us ACT slack of one (p, sp)
                work = []
                if 1 <= p and p + 2 < NPAIR:
                    if sp == 0:
                        kp2 = kpool.tile([P, S], BF16, tag="kp",
                                         name=f"kp_{p + 2}")
                        kps.append(kp2)
                        kplast = kp2
                        work += [lambda: qT_span(p + 2, 0, ppsL)]
                        work += [(lambda s=s, k=kp2: kT_span(k, p + 2, s, ppsL))
                                 for s in range(2)]
                    else:
                        work += [lambda: qT_span(p + 2, 1, ppsL)]
                        work += [(lambda s=s, k=kplast: kT_span(k, p + 2, s, ppsL))
                                 for s in range(2, NKS)]
                if sp == 0 and p == 0:
                    # last quarter of v(sp0): lands just ahead of its use
                    work += [(lambda i=i: v_span(i, 0, ppsL))
                             for i in range(3 * NS // 4, NS)]
                if NCS > 1 and (p, sp) in V1:
                    lo, hi = V1[(p, sp)]
                    work += [(lambda i=i: v_span(i, 1, ppsL))
                             for i in range(lo, hi)]
                if p == NPAIR - 1 and sp == 1:
                    work += [(lambda si=si, c=c: oproj_span(si, c, ppsL, ystg))
                             for si in range(Sq // (2 * P))
                             for c in range(NCS)]
                o_even = opsum.tile([W65, QSP], F32, tag="op",
                                    name=f"oe_{p}_{sp}")
                o_odd = opsum.tile([W65, QSP], F32, tag="op",
                                   name=f"oo_{p}_{sp}")
                for ktp in range(NSP):
                    if ktp == 0 and carry is not None:
                        expair = carry
                        carry = None
                        ts = (1,)          # (kt=0, t=0) already emitted
                    else:
                        expair = exq.tile([P, 2, 2 * QSP], AV_DT, tag="ex",
                                          name=f"ex_{p}_{sp}_{ktp}")
                        ts = (0, 1)
                    for t in ts:
                        emit_sc_act(kp, p, qsl, 2 * ktp + t, expair, t)
                    if DR_AV:
                        nc.tensor.matmul(
                            o_even,
                            v3[:, 2 * ktp:2 * ktp + 2, 2 * p, :],
                            expair[:, :, 0:QSP],
                            start=(ktp == 0), stop=(ktp == NSP - 1),
                            perf_mode=DR,
                        )
                        nc.tensor.matmul(
                            o_odd,
                            v3[:, 2 * ktp:2 * ktp + 2, 2 * p + 1, :],
                            expair[:, :, QSP:2 * QSP],
                            start=(ktp == 0), stop=(ktp == NSP - 1),
                            perf_mode=DR,
                        )
                    else:
                        for t in range(2):
                            nc.tensor.matmul(
                                o_even,
                                v3[:, 2 * ktp + t, 2 * p, :],
                                expair[:, t, 0:QSP],
                                start=(ktp == 0 and t == 0),
                                stop=(ktp == NSP - 1 and t == 1),
                            )
                            nc.tensor.matmul(
                                o_odd,
                                v3[:, 2 * ktp + t, 2 * p + 1, :],
                                expair[:, t, QSP:2 * QSP],
                                start=(ktp == 0 and t == 0),
                                stop=(ktp == NSP - 1 and t == 1),
                            )
                    if work:
                        work.pop(0)()
                # prologue of the next phase: its first scores+exp go ahead of
                # our epilogue so the scalar engine never idles at the boundary
                if idx + 1 < len(phases):
                    np_, nsp = phases[idx + 1]
                    nqsl = slice(nsp * QSP, (nsp + 1) * QSP)
                    carry = exq.tile([P, 2, 2 * QSP], AV_DT, tag="ex",
                                     name=f"ex_{np_}_{nsp}_0")
                    emit_sc_act(kps[np_], np_, nqsl, 0, carry, 0)
                # epilogue: oT = o_num * recip(den)  (= o + bv, scales cancel)
                for par, ops in ((0, o_even), (1, o_odd)):
                    den0 = eps.tile([1, QSP], F32, tag="den0")
                    nc.vector.tensor_copy(den0, ops[DH:W65, :])
                    rc0 = eps.tile([1, QSP], F32, tag="rc0")
                    nc.vector.reciprocal_approx_fast(rc0, den0)
                    rb = eps.tile([DH, QSP], F32, tag="rb_sb")
                    nc.gpsimd.partition_broadcast(rb, rc0, channels=DH)
                    if par == 0:
                        nc.vector.tensor_mul(
                            oT[0:DH, p, qsl], ops[0:DH, :], rb)
                    else:
                        on = eps.tile([DH, QSP], BF16, tag="on")
                        nc.vector.tensor_mul(on, ops[0:DH, :], rb)
                        nc.sync.dma_start(out=oT[DH:P, p, qsl], in_=on)
                    if work:
                        work.pop(0)()
                for w in work:
                    w()

        # ---- Output projection tail (second query half) ----
        with tc.tile_pool(name="ystg2", bufs=4) as ystg2, \
             tc.tile_pool(name="yps", bufs=2, space="PSUM") as yps:
            for sc_i in range(Sq // (2 * P), Sq // P):
                pss = [yps.tile([P, CSP], F32, tag=f"yp{csp}",
                                name=f"yps_{sc_i}_{csp}") for csp in range(NCS)]
                for j in range(ND):
                    for csp in range(NCS):
                        nc.tensor.matmul(
                            pss[csp],
                            oT[:, j, sc_i * P:(sc_i + 1) * P],
                            Wo_sb[:, j, csp * CSP:(csp + 1) * CSP],
                            start=(j == 0), stop=(j == ND - 1),
                        )
                for csp in range(NCS):
                    ysb = ystg2.tile([P, CSP], F32, tag="ysb")
                    nc.vector.tensor_add(ysb, pss[csp],
                                         bo_bc[:, csp * CSP:(csp + 1) * CSP])
                    nc.sync.dma_start(
                        out=y[sc_i * P:(sc_i + 1) * P, csp * CSP:(csp + 1) * CSP],
                        in_=ysb,
                    )

    nc.compile()
    return nc


_NC = None


def _get_nc():
    global _NC
    if _NC is None:
        _NC = build_mha_nc(S=S_FULL, Sq=S_FULL // 2, D=D_FULL, H=H_FULL)
    return _NC


def _to_fp8(a):
    return np.clip(np.asarray(a, np.float32), -240.0, 240.0).astype(NPFP8)


def shard_inputs(inputs):
    x = np.asarray(inputs["x"], dtype=np.float32)
    shared = {}
    for n, f in (("Wk", K_FP8), ("Wv", V_FP8), ("Wq", True)):
        w = 32.0 * np.asarray(inputs[n], dtype=np.float32)
        # W_sb[p, j, c] = 32*W[j*128+p, c], flattened to [128, 8*1024]
        w = _to_fp8(w) if f else w.astype(NPBF16)
        shared[n] = np.ascontiguousarray(
            w.reshape(8, P, D_FULL).transpose(1, 0, 2).reshape(P, -1))
    wo = np.asarray(inputs["Wo"], dtype=np.float32)
    shared["Wo"] = np.ascontiguousarray(
        wo.astype(NPBF16).reshape(8, P, D_FULL).transpose(1, 0, 2).reshape(P, -1))
    shared["bq"] = np.ascontiguousarray(32.0 * np.asarray(inputs["bq"], np.float32))
    shared["bk"] = np.ascontiguousarray(32.0 * np.asarray(inputs["bk"], np.float32))
    shared["bv"] = np.ascontiguousarray(32.0 * np.asarray(inputs["bv"], np.float32))
    shared["bo"] = np.ascontiguousarray(np.asarray(inputs["bo"], np.float32))
    half = S_FULL // 2
    maps = []
    for c in range(N_CORES):
        b, h = divmod(c, 2)
        xb = x[b]
        xp = np.concatenate([xb[h * half:(h + 1) * half],
                             xb[(1 - h) * half:(2 - h) * half]], axis=0)
        # xT[p, spn, j, u] = xp[spn*512+u, j*128+p]  (span-major per partition)
        xpT = xp.T
        xt = (_to_fp8(xpT).reshape(8, P, 4, 512)
              .transpose(1, 2, 0, 3).reshape(P, 4, 8, 512))
        xb_len = S_FULL // 2 if (K_FP8 and V_FP8) else S_FULL
        xtb = (xpT[:, :xb_len].astype(NPBF16)
               .reshape(8, P, xb_len // 512, 512)
               .transpose(1, 2, 0, 3).reshape(P, xb_len // 512, 8, 512))
        m = dict(shared)
        m["xT"] = np.ascontiguousarray(xt)
        m["xTb"] = np.ascontiguousarray(xtb)
        maps.append(m)
    return maps


def run(inputs, trace=False):
    nc = _get_nc()
    maps = shard_inputs(inputs)
    res = run_bass_kernel_spmd(nc, maps, list(range(N_CORES)), trace=trace)
    half = S_FULL // 2
    y = np.empty((B_FULL, S_FULL, D_FULL), dtype=np.float32)
    for c in range(N_CORES):
        b, h = divmod(c, 2)
        y[b, h * half:(h + 1) * half] = res.results[c]["y"]
    return y, res


def kernel(**inputs):
    y, _ = run(inputs, trace=False)
    return y
